# revision 59
# baseline (speedup 1.0000x reference)
"""Trainium2 Bass kernel for CausalSelfAttention (GQA + per-head RMS norm + RoPE).

Sharding: 8 cores = batch(2) x kv-head-group(4). Each core computes, for its
(b, g): qkv projection (its 4 rep q heads + 1 kv head), per-head RMS norm,
RoPE, causal attention, and a partial output projection (its 512 rows of
w_proj). Host sums the 4 partial projections per batch element.

Structure (v3):
  - x is fed as bf16 twice: natural rows (token rms-norm) and pre-transposed
    x^T (qkv matmul lhsT) — no on-chip x transposes at all.
  - All matmuls run bf16 x bf16 or f32r x f32r (1 cycle/row at free >= 256).
    q/k transposes use an f32r identity (1.5 c/r), evacuated as bf16.
  - Causal diagonal blocks are shrunk: score matmuls, exp, mask adds and
    the attn@v matmuls cover only the un-masked t-range of each s-tile
    (partial-width PSUM accumulation groups).
  - Token rms_norm(x) commutes out of q/k (they are re-normalized per head);
    only v is scaled by the per-token rstd(x).
  - All rstd values are computed as exp(-0.5*ln(ms+eps)) so the Scalar
    engine only ever uses the {ln, exp, square, copy} activation table —
    no ACT_TABLE_LOAD thrash between fused phases.
  - ScoresT[s, t] orientation: exp(scoresT) feeds attn@v directly and the
    output lands as aoT[d, t] = proj lhsT. Softmax denominator accumulates
    elementwise (DVE + Pool), partition-sums via a ones-column matmul, and
    the reciprocal is broadcast back with a ones-row matmul.
  - Fused schedule: one continuous qkv tile stream; attention chunk c is
    activated as soon as tile 4c+3 is emitted; projection chunk c follows
    its attention. A master weave interleaves the three streams to keep the
    PE dense (2.4 GHz p-state).
  - GpSimd (Pool) takes rope rotations, part of the denominator adds, and
    issues the y output DMAs (software DGE) so the SP queue never blocks
    input loads on compute.
"""

import os

import numpy as np
import ml_dtypes

from concourse import bacc, bass, mybir
from concourse import tile
from concourse.bass_utils import run_bass_kernel_spmd

# Problem shape (hardcoded per contract)
B, T, C = 2, 2048, 2048
N_HEADS, N_KV = 16, 4
HD = C // N_HEADS            # 128
REP = N_HEADS // N_KV        # 4
KV_DIM = N_KV * HD           # 512
P = 128
TT = T // P                  # 16 token tiles
KT = C // P                  # 16 contraction tiles
JQ = REP * HD                # 512 local q cols
JTOT = JQ + 2 * HD           # 768 local qkv cols
TCW = 512                    # attention t-chunk width
NTC = T // TCW               # 4
H2 = HD // 2
EPS = 1.1920929e-07
SCALE = 1.0 / float(np.sqrt(HD))
MASKVAL = -1.0e30

F32 = mybir.dt.float32
F32R = mybir.dt.float32r
BF16 = mybir.dt.bfloat16
AF = mybir.ActivationFunctionType


def _emit(nc):
    xb_d = nc.dram_tensor("xb", [T, C], BF16, kind="ExternalInput")
    xt_d = nc.dram_tensor("xt", [C, T], BF16, kind="ExternalInput")
    wqkv_d = nc.dram_tensor("wqkv", [C, JTOT], BF16, kind="ExternalInput")
    wproj_d = nc.dram_tensor("wproj", [JQ, C], BF16, kind="ExternalInput")
    gain_d = nc.dram_tensor("gain", [1, REP], F32, kind="ExternalInput")
    cos_d = nc.dram_tensor("costab", [T, HD], F32, kind="ExternalInput")
    sin_d = nc.dram_tensor("sintab", [T, HD], F32, kind="ExternalInput")  # [:, :64] = -sin
    mask_d = nc.dram_tensor("maskdiag", [P, P], F32, kind="ExternalInput")  # 0 / -1e30
    id_d = nc.dram_tensor("ident", [P, P], F32R, kind="ExternalInput")
    y_d = nc.dram_tensor("y", [T, C], F32, kind="ExternalOutput")

    with tile.TileContext(nc) as tc:
        with tc.tile_pool(name="persist", bufs=1) as pp, \
             tc.tile_pool(name="work", bufs=1) as wp, \
             tc.tile_pool(name="psum", bufs=1, space="PSUM") as psp:
            # ---- persistent activations / constants ----
            qT = [pp.tile([P, T], BF16, name=f"qT{h}", tag=f"qT{h}") for h in range(REP)]
            kTt = pp.tile([P, T], BF16, name="kTt", tag="kTt")
            vN = pp.tile([P, TT, HD], BF16, name="vN", tag="vN")

            wqkv_sb = pp.tile([P, KT, JTOT], BF16, name="wqkv_sb", tag="wqkv")
            wproj_sb = pp.tile([P, REP, C], BF16, name="wproj_sb", tag="wproj")
            wproj_loaded = [False]

            def load_wproj():
                wp4 = wproj_d.ap().rearrange("(h p) c -> p h c", p=P)
                for h in range(REP):
                    nc.sync.dma_start(out=wproj_sb[:, h:h + 1, :],
                                      in_=wp4[:, h:h + 1, :])
                wproj_loaded[0] = True

            mask_sb = pp.tile([P, P], F32, name="mask_sb", tag="mask")
            id_sb = pp.tile([P, P], F32R, name="id_sb", tag="ident")
            gainb = pp.tile([P, REP], F32, name="gainb", tag="gainb")

            def load_consts():
                nc.sync.dma_start(out=mask_sb, in_=mask_d.ap())
                nc.sync.dma_start(out=id_sb, in_=id_d.ap())
                nc.sync.dma_start(out=gainb,
                                  in_=gain_d.ap()[0].partition_broadcast(P))
            eps_t = pp.tile([P, 1], F32, name="eps_t", tag="eps")
            nc.vector.memset(eps_t, EPS)
            ones_f = pp.tile([P, 1], F32, name="ones_f", tag="ones_f")
            nc.vector.memset(ones_f, 1.0)
            ones_col = pp.tile([P, 1], F32R, name="ones_col", tag="ones_col")
            nc.vector.tensor_copy(ones_col, ones_f)
            onesr_f = pp.tile([1, P], F32, name="onesr_f", tag="onesr_f")
            nc.vector.memset(onesr_f, 1.0)
            ones_row = pp.tile([1, P], F32R, name="ones_row", tag="ones_row")
            nc.vector.tensor_copy(ones_row, onesr_f)

            cos4 = cos_d.ap().rearrange("(tt p) d -> p tt d", p=P)
            sin4 = sin_d.ap().rearrange("(tt p) d -> p tt d", p=P)

            xTc_tiles = {}
            ao_tiles = {}
            tiles_emitted = [0]
            prev_qk = []  # software-pipelined q/k transposes (2-tile delay)

            def emit_qk_transposes(qfr_t, kfr_t, ptt):
                tq = psp.tile([P, JQ], F32R, name=f"tq_{ptt}", tag="mm", bufs=3)
                for h in range(REP):
                    nc.tensor.transpose(tq[:, h * P:(h + 1) * P],
                                        qfr_t[:, h * P:(h + 1) * P], id_sb)
                for h in range(REP):
                    if h % 2 == 0:
                        nc.vector.tensor_copy(qT[h][:, ptt * P:(ptt + 1) * P],
                                              tq[:, h * P:(h + 1) * P].bitcast(F32))
                    else:
                        nc.scalar.copy(qT[h][:, ptt * P:(ptt + 1) * P],
                                       tq[:, h * P:(h + 1) * P].bitcast(F32))
                tk = psp.tile([P, HD], F32R, name=f"tk_{ptt}", tag="mm", bufs=3)
                nc.tensor.transpose(tk, kfr_t, id_sb)
                nc.vector.tensor_copy(kTt[:, ptt * P:(ptt + 1) * P], tk.bitcast(F32))

            I32 = mybir.dt.int32

            def quake_rsqrt(rstd6, ms6, scratch):
                """rstd6 = ms6^-0.5 elementwise: quake seed + 2 Newton steps."""
                mi = ms6.bitcast(I32)
                yi = scratch("qk_yi", I32)
                # ~(mi >> 1), then + (0x5f3759df + 1)  ==  0x5f3759df - (mi>>1)
                nc.vector.tensor_scalar(yi, mi, 1, 0xFFFFFFFF,
                                        mybir.AluOpType.logical_shift_right,
                                        mybir.AluOpType.bitwise_xor)
                nc.vector.tensor_scalar_add(yi, yi, 0x5f3759e0)
                y = yi.bitcast(F32)
                a = scratch("qk_a", F32)
                cfac = scratch("qk_c", F32)
                for it in range(2):
                    nc.vector.tensor_mul(a, ms6, y)
                    nc.vector.tensor_mul(a, a, y)
                    nc.vector.tensor_scalar(cfac, a, -0.5, 1.5,
                                            mybir.AluOpType.mult,
                                            mybir.AluOpType.add)
                    if it == 0:
                        nc.vector.tensor_mul(y, y, cfac)
                    else:
                        nc.vector.tensor_mul(rstd6, y, cfac)

            xt4 = xt_d.ap().rearrange("(kg k p) t -> p kg k t", p=P, kg=4)

            def load_xTc(tci):
                xTc = wp.tile([P, KT, TCW], BF16, name=f"xTc_{tci}",
                              tag="xTc", bufs=2)
                if tci == 0:
                    # fine-grained so tile 0's first contraction arrives fast
                    for kt in range(KT):
                        nc.sync.dma_start(
                            out=xTc[:, kt, :],
                            in_=xt_d.ap()[kt * P:(kt + 1) * P, 0:TCW])
                else:
                    # prefetched with ~8 tiles of slack: fewer, bigger DMAs
                    for kg in range(4):
                        nc.sync.dma_start(
                            out=xTc[:, kg * 4:(kg + 1) * 4, :],
                            in_=xt4[:, kg, :, tci * TCW:(tci + 1) * TCW])
                xTc_tiles[tci] = xTc

            # ----------------- phase-1 work for one token tile -------------
            cs_tiles = {}

            wq16 = wqkv_d.ap().rearrange("(kt p) j -> p kt j", p=P)

            def load_first_ops(xTc0, kt_lo, kt_hi):
                """Interleave x^T and wqkv per-kt: the kt-k operands of tile
                0's matmuls arrive in issue order, so the first matmul waits
                on DMA issue #2, not #23."""
                for kt in range(kt_lo, kt_hi):
                    nc.sync.dma_start(out=xTc0[:, kt, :],
                                      in_=xt_d.ap()[kt * P:(kt + 1) * P, 0:TCW])
                    nc.sync.dma_start(out=wqkv_sb[:, kt:kt + 1, :],
                                      in_=wq16[:, kt:kt + 1, :])

            def tile_work(tt):
                tci, tb = tt // 4, tt % 4
                if tt == 0:
                    xTc0 = wp.tile([P, KT, TCW], BF16, name="xTc_0",
                                   tag="xTc", bufs=2)
                    load_first_ops(xTc0, 0, 4)
                    xTc_tiles[0] = xTc0
                if tb == 1 and tci < NTC - 1:
                    load_xTc(tci + 1)  # prefetch next chunk
                xTc = xTc_tiles[tci]

                if tt % 2 == 0:
                    xr2 = wp.tile([P, 2, C], BF16, name=f"xr_{tt}", tag="xr", bufs=2)
                    nc.sync.dma_start(
                        out=xr2, in_=xb_d.ap()[tt * P:(tt + 2) * P, :]
                        .rearrange("(two p) c -> p two c", p=P))
                    cs_tiles["xr"] = xr2
                xr_t = cs_tiles["xr"][:, tt % 2, :]
                if tb == 0:
                    cosc = wp.tile([P, 4, HD], F32, name=f"cosc_{tci}",
                                   tag="cos", bufs=2)
                    nc.sync.dma_start(out=cosc, in_=cos4[:, tci * 4:tci * 4 + 4])
                    sinc = wp.tile([P, 4, HD], F32, name=f"sinc_{tci}",
                                   tag="sin", bufs=2)
                    nc.sync.dma_start(out=sinc, in_=sin4[:, tci * 4:tci * 4 + 4])
                    cs_tiles["cs"] = (cosc, sinc)
                cos_t = cs_tiles["cs"][0][:, tb, :]
                sin_t = cs_tiles["cs"][1][:, tb, :]
                if tt == 0:
                    load_consts()
                    load_first_ops(xTc_tiles[0], 4, KT)
                yield

                # x sum-of-squares (for v's token rstd) on ACT
                sx4 = wp.tile([P, 4], F32, name=f"sx4_{tt}", tag="sx4", bufs=2)
                for i in range(4):
                    scr = wp.tile([P, TCW], F32, name=f"scrx_{tt}_{i}",
                                  tag="scr", bufs=2)
                    nc.scalar.activation(scr, xr_t[:, i * TCW:(i + 1) * TCW],
                                         AF.Square, accum_out=sx4[:, i:i + 1])
                sums6 = wp.tile([P, 6], F32, name=f"sums6_{tt}", tag="sums6", bufs=2)
                nc.vector.reduce_sum(sums6[:, 0:1], sx4, axis=mybir.AxisListType.X)
                yield

                # qkv matmuls (bf16 x bf16)
                q_ps = psp.tile([P, JQ], F32, name=f"qps_{tt}", tag="qps", bufs=2)
                kv_ps = psp.tile([P, 2 * HD], F32, name=f"kvps_{tt}", tag="kv", bufs=1)
                for kt in range(KT):
                    lb = xTc[:, kt, tb * P:(tb + 1) * P]
                    nc.tensor.matmul(q_ps, lb, wqkv_sb[:, kt, 0:JQ],
                                     start=(kt == 0), stop=(kt == KT - 1))
                    nc.tensor.matmul(kv_ps, lb, wqkv_sb[:, kt, JQ:JTOT],
                                     start=(kt == 0), stop=(kt == KT - 1))
                    if kt % 4 == 3:
                        yield

                # q/k transposes from two tiles back: their rope finished while
                # two tiles' worth of qkv matmuls ran, so the PE never waits
                if len(prev_qk) >= 2:
                    args = prev_qk.pop(0)
                    emit_qk_transposes(*args)
                    tiles_emitted[0] = args[2] + 1
                    yield

                # q/k sums of squares into sums6[:,1:6]; one quake rsqrt for
                # all six rstds (x, q0..q3, k) — no Sqrt/Ln on ACT, so the
                # scalar engine never reloads its activation table.
                for h in range(REP):
                    scr = wp.tile([P, TCW], F32, name=f"scrq_{tt}_{h}",
                                  tag="scr", bufs=2)
                    nc.scalar.activation(scr[:, :HD], q_ps[:, h * HD:(h + 1) * HD],
                                         AF.Square, accum_out=sums6[:, 1 + h:2 + h])
                scrk = wp.tile([P, TCW], F32, name=f"scrk_{tt}", tag="scr", bufs=2)
                nc.scalar.activation(scrk[:, :HD], kv_ps[:, 0:HD], AF.Square,
                                     accum_out=sums6[:, 5:6])
                ms6 = wp.tile([P, 6], F32, name=f"ms6_{tt}", tag="ms6", bufs=2)
                nc.vector.tensor_scalar(ms6[:, 0:1], sums6[:, 0:1], 1.0 / C, EPS,
                                        mybir.AluOpType.mult, mybir.AluOpType.add)
                nc.vector.tensor_scalar(ms6[:, 1:6], sums6[:, 1:6], 1.0 / HD, EPS,
                                        mybir.AluOpType.mult, mybir.AluOpType.add)
                rstd6 = wp.tile([P, 6], F32, name=f"rstd6_{tt}", tag="rstd6", bufs=2)

                def scratch(nm, dt, tt=tt):
                    return wp.tile([P, 6], dt, name=f"{nm}_{tt}", tag=nm, bufs=2)

                quake_rsqrt(rstd6, ms6, scratch)
                rstdx = rstd6[:, 0:1]
                rstdk = rstd6[:, 5:6]
                rstdqg = wp.tile([P, REP], F32, name=f"rstdqg_{tt}", tag="rstdqg", bufs=2)
                nc.vector.tensor_mul(rstdqg, rstd6[:, 1:5], gainb)
                yield

                qn_t = wp.tile([P, JQ], F32, name=f"qn_{tt}", tag="qn", bufs=2)
                qn3 = qn_t.rearrange("p (h d) -> p h d", h=REP)
                nc.vector.tensor_mul(
                    qn3, q_ps.rearrange("p (h d) -> p h d", h=REP),
                    rstdqg[:, :, None].broadcast_to([P, REP, HD]))
                # rope q: qf = qn*cos + rot(qn)*sin   (rot halves on Pool)
                qB_t = wp.tile([P, JQ], F32, name=f"qB_{tt}", tag="qB", bufs=2)
                qB3 = qB_t.rearrange("p (h d) -> p h d", h=REP)
                nc.gpsimd.tensor_mul(qB3[:, :, 0:H2], qn3[:, :, H2:HD],
                                     sin_t[:, None, 0:H2].broadcast_to([P, REP, H2]))
                nc.gpsimd.tensor_mul(qB3[:, :, H2:HD], qn3[:, :, 0:H2],
                                     sin_t[:, None, H2:HD].broadcast_to([P, REP, H2]))
                qf_t = wp.tile([P, JQ], F32, name=f"qf_{tt}", tag="qf", bufs=2)
                qf3 = qf_t.rearrange("p (h d) -> p h d", h=REP)
                nc.vector.tensor_mul(qf3, qn3,
                                     cos_t[:, None, :].broadcast_to([P, REP, HD]))
                qfr_t = wp.tile([P, JQ], F32R, name=f"qfr_{tt}", tag="qfr", bufs=3)
                nc.gpsimd.tensor_add(qfr_t, qf_t, qB_t)
                yield

                # k: rms norm + rope (rope on Pool)
                kn_t = wp.tile([P, HD], F32, name=f"kn_{tt}", tag="kn", bufs=2)
                nc.vector.tensor_scalar_mul(kn_t, kv_ps[:, 0:HD], rstdk)
                kB_t = wp.tile([P, HD], F32, name=f"kB_{tt}", tag="kB", bufs=2)
                nc.gpsimd.tensor_mul(kB_t[:, 0:H2], kn_t[:, H2:HD], sin_t[:, 0:H2])
                nc.gpsimd.tensor_mul(kB_t[:, H2:HD], kn_t[:, 0:H2], sin_t[:, H2:HD])
                kf_t = wp.tile([P, HD], F32, name=f"kf_{tt}", tag="kf", bufs=2)
                nc.gpsimd.tensor_mul(kf_t, kn_t, cos_t)
                kfr_t = wp.tile([P, HD], F32R, name=f"kfr_{tt}", tag="kfr", bufs=3)
                nc.gpsimd.tensor_add(kfr_t, kf_t, kB_t)
                # v: scale rows by token rstd
                with nc.allow_low_precision(reason="bf16 matmul operand"):
                    nc.vector.tensor_scalar_mul(vN[:, tt, :], kv_ps[:, HD:2 * HD],
                                                rstdx)
                yield

                prev_qk.append((qfr_t, kfr_t, tt))
                yield

            # ----------------- attention for one (chunk, head) --------------
            def attend(c, h, o_ps):
                nst = 4 * (c + 1)
                denf_a = wp.tile([P, TCW], F32R, name=f"dna_{c}_{h}",
                                 tag="dena", bufs=4)
                denf_b = None
                if c >= 1:
                    denf_b = wp.tile([P, TCW], F32R, name=f"dnb_{c}_{h}",
                                     tag="denb", bufs=4)
                for st in range(nst):
                    dv = st - 4 * c
                    off = dv * P if dv >= 0 else 0
                    w = TCW - off
                    sc = psp.tile([P, w], F32, name=f"sc_{c}_{h}_{st}",
                                  tag="mm", bufs=3)
                    nc.tensor.matmul(sc, kTt[:, st * P:(st + 1) * P],
                                     qT[h][:, c * TCW + off:(c + 1) * TCW],
                                     start=True, stop=True)
                    if dv >= 0:
                        nc.vector.tensor_add(sc[:, 0:P], sc[:, 0:P], mask_sb)
                    et = wp.tile([P, w], BF16, name=f"et_{c}_{h}_{st}",
                                 tag="et", bufs=8)
                    nc.scalar.activation(et, sc, AF.Exp, scale=SCALE)
                    if st == 0:
                        nc.vector.tensor_copy(denf_a, et)
                    elif c >= 1 and st == 1:
                        nc.vector.tensor_copy(denf_b, et)
                    elif c >= 1 and st % 2 == 1:
                        nc.gpsimd.tensor_add(denf_b[:, off:TCW],
                                             denf_b[:, off:TCW], et)
                    else:
                        nc.vector.tensor_add(denf_a[:, off:TCW],
                                             denf_a[:, off:TCW], et)
                    nc.tensor.matmul(o_ps[:, off:TCW], vN[:, st, :], et,
                                     start=(st == 0), stop=(st == nst - 1),
                                     skip_group_check=True)
                    yield
                # denominator: partition-sum both partials on the PE directly
                ds_ps = psp.tile([1, TCW], F32, name=f"ds_{c}_{h}",
                                 tag="mm", bufs=3)
                nc.tensor.matmul(ds_ps, ones_col, denf_a,
                                 start=True, stop=(c == 0))
                if c >= 1:
                    nc.tensor.matmul(ds_ps, ones_col, denf_b,
                                     start=False, stop=True)
                dsum = wp.tile([1, TCW], F32R, name=f"dsum_{c}_{h}",
                               tag="dsum", bufs=2)
                with nc.allow_low_precision(reason="fp32r matmul operand"):
                    nc.scalar.copy(dsum, ds_ps)
                rb_ps = psp.tile([P, TCW], F32, name=f"rb_{c}_{h}",
                                 tag="mm", bufs=3)
                nc.tensor.matmul(rb_ps, ones_row, dsum, start=True, stop=True)
                rb = wp.tile([P, TCW], F32, name=f"rbs_{c}_{h}", tag="rb", bufs=2)
                nc.vector.reciprocal_approx_fast(rb, rb_ps)
                aot = wp.tile([P, TCW], BF16, name=f"ao_{c}_{h}", tag="ao", bufs=8)
                with nc.allow_low_precision(reason="bf16 matmul operand"):
                    nc.vector.tensor_mul(aot, o_ps, rb)
                ao_tiles[(c, h)] = aot
                yield

            def chunk_gen(c):
                """Attention for chunk c: two head-pairs, interleaved. The
                last chunk runs all four heads concurrently, borrowing the
                q_ps PSUM banks that the finished qkv tile stream no longer
                needs."""
                if not wproj_loaded[0]:
                    load_wproj()
                if c == NTC - 1:
                    gens = []
                    for h in range(REP):
                        o = psp.tile([P, TCW], F32, name=f"ops_{c}_{h}",
                                     tag="acc" if h < 2 else "qps", bufs=2)
                        gens.append(attend(c, h, o))
                    alive = True
                    while alive:
                        alive = False
                        for g in gens:
                            try:
                                next(g)
                                alive = True
                            except StopIteration:
                                pass
                        yield
                    return
                for hp in (0, 2):
                    o0 = psp.tile([P, TCW], F32, name=f"ops_{c}_{hp}",
                                  tag="acc", bufs=2)
                    o1 = psp.tile([P, TCW], F32, name=f"ops_{c}_{hp + 1}",
                                  tag="acc", bufs=2)
                    g0 = attend(c, hp, o0)
                    g1 = attend(c, hp + 1, o1)
                    alive = True
                    while alive:
                        alive = False
                        for g in (g0, g1):
                            try:
                                next(g)
                                alive = True
                            except StopIteration:
                                pass
                        yield

            def proj_steps(c):
                """Projection for chunk c as small emit-steps (4 MMs each)."""
                for ttl in range(4):
                    yt = wp.tile([P, C], F32, name=f"y_{c}_{ttl}", tag="y", bufs=2)
                    for ncs in range(4):
                        y_ps = psp.tile([P, TCW], F32,
                                        name=f"yps_{c}_{ttl}_{ncs}",
                                        tag="mm", bufs=3)
                        for h in range(REP):
                            nc.tensor.matmul(
                                y_ps,
                                ao_tiles[(c, h)][:, ttl * P:(ttl + 1) * P],
                                wproj_sb[:, h, ncs * TCW:(ncs + 1) * TCW],
                                start=(h == 0), stop=(h == REP - 1))
                        # ACT helps only while qkv tiles still run; in the tail
                        # the scalar engine is exp-critical, so DVE evacuates
                        if c < 2 and ncs % 2 == 1:
                            nc.scalar.copy(yt[:, ncs * TCW:(ncs + 1) * TCW], y_ps)
                        else:
                            nc.vector.tensor_copy(yt[:, ncs * TCW:(ncs + 1) * TCW],
                                                  y_ps)
                        yield
                    row = (c * 4 + ttl) * P
                    nc.sync.dma_start(out=y_d.ap()[row:row + P, :], in_=yt)
                    yield

            # ----------------- master weave ---------------------------------
            def drain(g, n):
                for _ in range(n):
                    try:
                        next(g)
                    except StopIteration:
                        return False
                return True

            def tiles_stream():
                for tt in range(TT):
                    yield from tile_work(tt)
                while prev_qk:
                    args = prev_qk.pop(0)
                    emit_qk_transposes(*args)
                    tiles_emitted[0] = args[2] + 1
                    yield

            ts = tiles_stream()
            ts_alive = True
            attn_q = [0, 1, 2, 3]
            cur_attn = None
            cur_attn_c = None
            proj_q = []
            cur_proj = None
            while ts_alive or cur_attn or attn_q or cur_proj or proj_q:
                if ts_alive:
                    ts_alive = drain(ts, 2)
                if cur_attn is None and attn_q and \
                        tiles_emitted[0] >= 4 * (attn_q[0] + 1):
                    cur_attn_c = attn_q.pop(0)
                    cur_attn = chunk_gen(cur_attn_c)
                if cur_attn is not None:
                    if not drain(cur_attn, 2):
                        proj_q.append(cur_attn_c)
                        cur_attn = None
                if cur_proj is None and proj_q:
                    cur_proj = proj_steps(proj_q.pop(0))
                if cur_proj is not None:
                    if not drain(cur_proj, 1):
                        cur_proj = None

    return nc


_NC_CACHE = {}
LAST_RESULT = None


def _get_nc():
    if "v3" not in _NC_CACHE:
        nc = bacc.Bacc("TRN2", target_bir_lowering=False, debug=False)
        _emit(nc)
        nc.compile()
        _NC_CACHE["v3"] = nc
    return _NC_CACHE["v3"]


def _host_tables():
    inv_freq = 1.0 / (10000.0 ** (np.arange(0, HD, 2, dtype=np.float64) / HD))
    t = np.arange(T, dtype=np.float64)
    freqs = np.outer(t, inv_freq)                      # [T, 64]
    emb = np.concatenate([freqs, freqs], axis=-1)      # [T, 128]
    cos = np.cos(emb).astype(np.float32)
    sin = np.sin(emb).astype(np.float32)
    sin_signed = sin.copy()
    sin_signed[:, :HD // 2] *= -1.0                    # first half gets -sin
    # single diagonal-block causal mask, scoresT layout:
    # mask[s, t] = 0 if s <= t else -1e30 (within a 128x128 diagonal block)
    s = np.arange(P)[:, None]
    tcol = np.arange(P)[None, :]
    mask = np.where(s <= tcol, 0.0, MASKVAL).astype(np.float32)
    ident = np.eye(P, dtype=np.float32)
    return cos, sin_signed, mask, ident


def kernel(x, w_qkv, w_proj, q_gain):
    global LAST_RESULT
    x = np.asarray(x, dtype=np.float32)
    w_qkv = np.asarray(w_qkv, dtype=np.float32)
    w_proj = np.asarray(w_proj, dtype=np.float32)
    q_gain = np.asarray(q_gain, dtype=np.float32)

    cos, sin_signed, mask, ident = _host_tables()
    nc = _get_nc()

    x_bf = [np.ascontiguousarray(x[b].astype(ml_dtypes.bfloat16))
            for b in range(B)]
    xt_bf = [np.ascontiguousarray(x_bf[b].T) for b in range(B)]
    in_maps = []
    for r in range(8):
        b, g = r // 4, r % 4
        wq = w_qkv[:, g * JQ:(g + 1) * JQ]
        wk = w_qkv[:, C + g * HD:C + (g + 1) * HD]
        wv = w_qkv[:, C + KV_DIM + g * HD:C + KV_DIM + (g + 1) * HD]
        in_maps.append({
            "xb": x_bf[b],
            "xt": xt_bf[b],
            "wqkv": np.ascontiguousarray(
                np.concatenate([wq, wk, wv], axis=1)).astype(ml_dtypes.bfloat16),
            "wproj": np.ascontiguousarray(
                w_proj[g * JQ:(g + 1) * JQ, :]).astype(ml_dtypes.bfloat16),
            "gain": np.ascontiguousarray(q_gain[g * REP:(g + 1) * REP].reshape(1, REP)),
            "costab": cos,
            "sintab": sin_signed,
            "maskdiag": mask,
            "ident": ident,
        })

    trace = os.environ.get("KERNEL_TRACE") == "1"
    if trace:
        try:
            import antenv.axon_hooks  # noqa: F401
        except ImportError:
            trace = False
    res = run_bass_kernel_spmd(nc, in_maps, core_ids=list(range(8)), trace=trace)
    LAST_RESULT = res

    out = np.zeros((B, T, C), dtype=np.float32)
    for r in range(8):
        b = r // 4
        out[b] += res.results[r]["y"]
    return out


# revision 60
# speedup vs baseline: 1.0200x; 1.0200x over previous
"""Trainium2 Bass kernel for CausalSelfAttention (GQA + per-head RMS norm + RoPE).

Sharding: 8 cores = batch(2) x kv-head-group(4). Each core computes, for its
(b, g): qkv projection (its 4 rep q heads + 1 kv head), per-head RMS norm,
RoPE, causal attention, and a partial output projection (its 512 rows of
w_proj). Host sums the 4 partial projections per batch element.

Structure (v3):
  - x is fed as bf16 twice: natural rows (token rms-norm) and pre-transposed
    x^T (qkv matmul lhsT) — no on-chip x transposes at all.
  - All matmuls run bf16 x bf16 or f32r x f32r (1 cycle/row at free >= 256).
    q/k transposes use an f32r identity (1.5 c/r), evacuated as bf16.
  - Causal diagonal blocks are shrunk: score matmuls, exp, mask adds and
    the attn@v matmuls cover only the un-masked t-range of each s-tile
    (partial-width PSUM accumulation groups).
  - Token rms_norm(x) commutes out of q/k (they are re-normalized per head);
    only v is scaled by the per-token rstd(x).
  - All rstd values are computed as exp(-0.5*ln(ms+eps)) so the Scalar
    engine only ever uses the {ln, exp, square, copy} activation table —
    no ACT_TABLE_LOAD thrash between fused phases.
  - ScoresT[s, t] orientation: exp(scoresT) feeds attn@v directly and the
    output lands as aoT[d, t] = proj lhsT. Softmax denominator accumulates
    elementwise (DVE + Pool), partition-sums via a ones-column matmul, and
    the reciprocal is broadcast back with a ones-row matmul.
  - Fused schedule: one continuous qkv tile stream; attention chunk c is
    activated as soon as tile 4c+3 is emitted; projection chunk c follows
    its attention. A master weave interleaves the three streams to keep the
    PE dense (2.4 GHz p-state).
  - GpSimd (Pool) takes rope rotations, part of the denominator adds, and
    issues the y output DMAs (software DGE) so the SP queue never blocks
    input loads on compute.
"""

import os

import numpy as np
import ml_dtypes

from concourse import bacc, bass, mybir
from concourse import tile
from concourse.bass_utils import run_bass_kernel_spmd

# Problem shape (hardcoded per contract)
B, T, C = 2, 2048, 2048
N_HEADS, N_KV = 16, 4
HD = C // N_HEADS            # 128
REP = N_HEADS // N_KV        # 4
KV_DIM = N_KV * HD           # 512
P = 128
TT = T // P                  # 16 token tiles
KT = C // P                  # 16 contraction tiles
JQ = REP * HD                # 512 local q cols
JTOT = JQ + 2 * HD           # 768 local qkv cols
TCW = 512                    # attention t-chunk width
NTC = T // TCW               # 4
H2 = HD // 2
EPS = 1.1920929e-07
SCALE = 1.0 / float(np.sqrt(HD))
MASKVAL = -1.0e30

F32 = mybir.dt.float32
F32R = mybir.dt.float32r
BF16 = mybir.dt.bfloat16
AF = mybir.ActivationFunctionType


def _emit(nc):
    xb_d = nc.dram_tensor("xb", [T, C], BF16, kind="ExternalInput")
    xt_d = nc.dram_tensor("xt", [C, T], BF16, kind="ExternalInput")
    wqkv_d = nc.dram_tensor("wqkv", [C, JTOT], BF16, kind="ExternalInput")
    wproj_d = nc.dram_tensor("wproj", [JQ, C], BF16, kind="ExternalInput")
    gain_d = nc.dram_tensor("gain", [1, REP], F32, kind="ExternalInput")
    cos_d = nc.dram_tensor("costab", [T, HD], F32, kind="ExternalInput")
    sin_d = nc.dram_tensor("sintab", [T, HD], F32, kind="ExternalInput")  # [:, :64] = -sin
    mask_d = nc.dram_tensor("maskdiag", [P, P], F32, kind="ExternalInput")  # 0 / -1e30
    id_d = nc.dram_tensor("ident", [P, P], F32R, kind="ExternalInput")
    y_d = nc.dram_tensor("y", [T, C], F32, kind="ExternalOutput")

    with tile.TileContext(nc) as tc:
        with tc.tile_pool(name="persist", bufs=1) as pp, \
             tc.tile_pool(name="work", bufs=1) as wp, \
             tc.tile_pool(name="psum", bufs=1, space="PSUM") as psp:
            # ---- persistent activations / constants ----
            qT = [pp.tile([P, T], BF16, name=f"qT{h}", tag=f"qT{h}") for h in range(REP)]
            kTt = pp.tile([P, T], BF16, name="kTt", tag="kTt")
            vN = pp.tile([P, TT, HD], BF16, name="vN", tag="vN")

            wqkv_sb = pp.tile([P, KT, JTOT], BF16, name="wqkv_sb", tag="wqkv")
            wproj_sb = pp.tile([P, REP, C], BF16, name="wproj_sb", tag="wproj")
            wproj_loaded = [False]

            def load_wproj():
                wp4 = wproj_d.ap().rearrange("(h p) c -> p h c", p=P)
                for h in range(REP):
                    nc.sync.dma_start(out=wproj_sb[:, h:h + 1, :],
                                      in_=wp4[:, h:h + 1, :])
                wproj_loaded[0] = True

            mask_sb = pp.tile([P, P], F32, name="mask_sb", tag="mask")
            id_sb = pp.tile([P, P], F32R, name="id_sb", tag="ident")
            gainb = pp.tile([P, REP], F32, name="gainb", tag="gainb")

            def load_consts():
                nc.sync.dma_start(out=mask_sb, in_=mask_d.ap())
                nc.sync.dma_start(out=id_sb, in_=id_d.ap())
                nc.sync.dma_start(out=gainb,
                                  in_=gain_d.ap()[0].partition_broadcast(P))
            eps_t = pp.tile([P, 1], F32, name="eps_t", tag="eps")
            nc.vector.memset(eps_t, EPS)
            ones_f = pp.tile([P, 1], F32, name="ones_f", tag="ones_f")
            nc.vector.memset(ones_f, 1.0)
            ones_col = pp.tile([P, 1], F32R, name="ones_col", tag="ones_col")
            nc.vector.tensor_copy(ones_col, ones_f)
            onesr_f = pp.tile([1, P], F32, name="onesr_f", tag="onesr_f")
            nc.vector.memset(onesr_f, 1.0)
            ones_row = pp.tile([1, P], F32R, name="ones_row", tag="ones_row")
            nc.vector.tensor_copy(ones_row, onesr_f)

            cos4 = cos_d.ap().rearrange("(tt p) d -> p tt d", p=P)
            sin4 = sin_d.ap().rearrange("(tt p) d -> p tt d", p=P)

            xTc_tiles = {}
            ao_tiles = {}
            tiles_emitted = [0]
            prev_qk = []  # software-pipelined q/k transposes (2-tile delay)

            def emit_qk_transposes(qfr_t, kfr_t, ptt):
                tq = psp.tile([P, JQ], F32R, name=f"tq_{ptt}", tag="mm", bufs=3)
                for h in range(REP):
                    nc.tensor.transpose(tq[:, h * P:(h + 1) * P],
                                        qfr_t[:, h * P:(h + 1) * P], id_sb)
                for h in range(REP):
                    if h % 2 == 0:
                        nc.vector.tensor_copy(qT[h][:, ptt * P:(ptt + 1) * P],
                                              tq[:, h * P:(h + 1) * P].bitcast(F32))
                    else:
                        nc.scalar.copy(qT[h][:, ptt * P:(ptt + 1) * P],
                                       tq[:, h * P:(h + 1) * P].bitcast(F32))
                tk = psp.tile([P, HD], F32R, name=f"tk_{ptt}", tag="mm", bufs=3)
                nc.tensor.transpose(tk, kfr_t, id_sb)
                nc.vector.tensor_copy(kTt[:, ptt * P:(ptt + 1) * P], tk.bitcast(F32))

            I32 = mybir.dt.int32

            def quake_rsqrt(rstd6, ms6, scratch):
                """rstd6 = ms6^-0.5 elementwise: quake seed + 2 Newton steps."""
                mi = ms6.bitcast(I32)
                yi = scratch("qk_yi", I32)
                # ~(mi >> 1), then + (0x5f3759df + 1)  ==  0x5f3759df - (mi>>1)
                nc.vector.tensor_scalar(yi, mi, 1, 0xFFFFFFFF,
                                        mybir.AluOpType.logical_shift_right,
                                        mybir.AluOpType.bitwise_xor)
                nc.vector.tensor_scalar_add(yi, yi, 0x5f3759e0)
                y = yi.bitcast(F32)
                a = scratch("qk_a", F32)
                cfac = scratch("qk_c", F32)
                for it in range(2):
                    nc.vector.tensor_mul(a, ms6, y)
                    nc.vector.tensor_mul(a, a, y)
                    nc.vector.tensor_scalar(cfac, a, -0.5, 1.5,
                                            mybir.AluOpType.mult,
                                            mybir.AluOpType.add)
                    if it == 0:
                        nc.vector.tensor_mul(y, y, cfac)
                    else:
                        nc.vector.tensor_mul(rstd6, y, cfac)

            xt4 = xt_d.ap().rearrange("(kg k p) t -> p kg k t", p=P, kg=4)

            def load_xTc(tci):
                xTc = wp.tile([P, KT, TCW], BF16, name=f"xTc_{tci}",
                              tag="xTc", bufs=2)
                if tci == 0:
                    # fine-grained so tile 0's first contraction arrives fast
                    for kt in range(KT):
                        nc.sync.dma_start(
                            out=xTc[:, kt, :],
                            in_=xt_d.ap()[kt * P:(kt + 1) * P, 0:TCW])
                else:
                    # prefetched with ~8 tiles of slack: fewer, bigger DMAs
                    for kg in range(4):
                        nc.sync.dma_start(
                            out=xTc[:, kg * 4:(kg + 1) * 4, :],
                            in_=xt4[:, kg, :, tci * TCW:(tci + 1) * TCW])
                xTc_tiles[tci] = xTc

            # ----------------- phase-1 work for one token tile -------------
            cs_tiles = {}

            wq16 = wqkv_d.ap().rearrange("(kt p) j -> p kt j", p=P)

            def load_first_ops(xTc0, kt_lo, kt_hi):
                """Interleave x^T and wqkv per-kt: the kt-k operands of tile
                0's matmuls arrive in issue order, so the first matmul waits
                on DMA issue #2, not #23."""
                for kt in range(kt_lo, kt_hi):
                    nc.sync.dma_start(out=xTc0[:, kt, :],
                                      in_=xt_d.ap()[kt * P:(kt + 1) * P, 0:TCW])
                    nc.sync.dma_start(out=wqkv_sb[:, kt:kt + 1, :],
                                      in_=wq16[:, kt:kt + 1, :])

            def tile_work(tt):
                tci, tb = tt // 4, tt % 4
                if tt == 0:
                    xTc0 = wp.tile([P, KT, TCW], BF16, name="xTc_0",
                                   tag="xTc", bufs=2)
                    load_first_ops(xTc0, 0, 4)
                    xTc_tiles[0] = xTc0
                if tb == 1 and tci < NTC - 1:
                    load_xTc(tci + 1)  # prefetch next chunk
                xTc = xTc_tiles[tci]

                if tt % 2 == 0:
                    xr2 = wp.tile([P, 2, C], BF16, name=f"xr_{tt}", tag="xr", bufs=2)
                    nc.sync.dma_start(
                        out=xr2, in_=xb_d.ap()[tt * P:(tt + 2) * P, :]
                        .rearrange("(two p) c -> p two c", p=P))
                    cs_tiles["xr"] = xr2
                xr_t = cs_tiles["xr"][:, tt % 2, :]
                if tb == 0:
                    cosc = wp.tile([P, 4, HD], F32, name=f"cosc_{tci}",
                                   tag="cos", bufs=2)
                    nc.sync.dma_start(out=cosc, in_=cos4[:, tci * 4:tci * 4 + 4])
                    sinc = wp.tile([P, 4, HD], F32, name=f"sinc_{tci}",
                                   tag="sin", bufs=2)
                    nc.sync.dma_start(out=sinc, in_=sin4[:, tci * 4:tci * 4 + 4])
                    cs_tiles["cs"] = (cosc, sinc)
                cos_t = cs_tiles["cs"][0][:, tb, :]
                sin_t = cs_tiles["cs"][1][:, tb, :]
                if tt == 0:
                    load_consts()
                    load_first_ops(xTc_tiles[0], 4, KT)
                yield

                # x sum-of-squares (for v's token rstd) on ACT
                sx4 = wp.tile([P, 4], F32, name=f"sx4_{tt}", tag="sx4", bufs=2)
                for i in range(4):
                    scr = wp.tile([P, TCW], F32, name=f"scrx_{tt}_{i}",
                                  tag="scr", bufs=2)
                    nc.scalar.activation(scr, xr_t[:, i * TCW:(i + 1) * TCW],
                                         AF.Square, accum_out=sx4[:, i:i + 1])
                sums6 = wp.tile([P, 6], F32, name=f"sums6_{tt}", tag="sums6", bufs=2)
                nc.vector.reduce_sum(sums6[:, 0:1], sx4, axis=mybir.AxisListType.X)
                yield

                # qkv matmuls (bf16 x bf16)
                q_ps = psp.tile([P, JQ], F32, name=f"qps_{tt}", tag="qps", bufs=2)
                kv_ps = psp.tile([P, 2 * HD], F32, name=f"kvps_{tt}", tag="kv", bufs=1)
                for kt in range(KT):
                    lb = xTc[:, kt, tb * P:(tb + 1) * P]
                    nc.tensor.matmul(q_ps, lb, wqkv_sb[:, kt, 0:JQ],
                                     start=(kt == 0), stop=(kt == KT - 1))
                    nc.tensor.matmul(kv_ps, lb, wqkv_sb[:, kt, JQ:JTOT],
                                     start=(kt == 0), stop=(kt == KT - 1))
                    if kt % 4 == 3:
                        yield

                # q/k transposes from two tiles back: their rope finished while
                # two tiles' worth of qkv matmuls ran, so the PE never waits
                if len(prev_qk) >= 2:
                    args = prev_qk.pop(0)
                    emit_qk_transposes(*args)
                    tiles_emitted[0] = args[2] + 1
                    yield

                # q/k sums of squares into sums6[:,1:6]; one quake rsqrt for
                # all six rstds (x, q0..q3, k) — no Sqrt/Ln on ACT, so the
                # scalar engine never reloads its activation table.
                for h in range(REP):
                    scr = wp.tile([P, TCW], F32, name=f"scrq_{tt}_{h}",
                                  tag="scr", bufs=2)
                    nc.scalar.activation(scr[:, :HD], q_ps[:, h * HD:(h + 1) * HD],
                                         AF.Square, accum_out=sums6[:, 1 + h:2 + h])
                scrk = wp.tile([P, TCW], F32, name=f"scrk_{tt}", tag="scr", bufs=2)
                nc.scalar.activation(scrk[:, :HD], kv_ps[:, 0:HD], AF.Square,
                                     accum_out=sums6[:, 5:6])
                ms6 = wp.tile([P, 6], F32, name=f"ms6_{tt}", tag="ms6", bufs=2)
                nc.vector.tensor_scalar(ms6[:, 0:1], sums6[:, 0:1], 1.0 / C, EPS,
                                        mybir.AluOpType.mult, mybir.AluOpType.add)
                nc.vector.tensor_scalar(ms6[:, 1:6], sums6[:, 1:6], 1.0 / HD, EPS,
                                        mybir.AluOpType.mult, mybir.AluOpType.add)
                rstd6 = wp.tile([P, 6], F32, name=f"rstd6_{tt}", tag="rstd6", bufs=2)

                def scratch(nm, dt, tt=tt):
                    return wp.tile([P, 6], dt, name=f"{nm}_{tt}", tag=nm, bufs=2)

                quake_rsqrt(rstd6, ms6, scratch)
                rstdx = rstd6[:, 0:1]
                rstdk = rstd6[:, 5:6]
                rstdqg = wp.tile([P, REP], F32, name=f"rstdqg_{tt}", tag="rstdqg", bufs=2)
                nc.vector.tensor_mul(rstdqg, rstd6[:, 1:5], gainb)
                yield

                qn_t = wp.tile([P, JQ], F32, name=f"qn_{tt}", tag="qn", bufs=2)
                qn3 = qn_t.rearrange("p (h d) -> p h d", h=REP)
                nc.vector.tensor_mul(
                    qn3, q_ps.rearrange("p (h d) -> p h d", h=REP),
                    rstdqg[:, :, None].broadcast_to([P, REP, HD]))
                # rope q: qf = qn*cos + rot(qn)*sin   (rot halves on Pool)
                qB_t = wp.tile([P, JQ], F32, name=f"qB_{tt}", tag="qB", bufs=2)
                qB3 = qB_t.rearrange("p (h d) -> p h d", h=REP)
                nc.gpsimd.tensor_mul(qB3[:, :, 0:H2], qn3[:, :, H2:HD],
                                     sin_t[:, None, 0:H2].broadcast_to([P, REP, H2]))
                nc.gpsimd.tensor_mul(qB3[:, :, H2:HD], qn3[:, :, 0:H2],
                                     sin_t[:, None, H2:HD].broadcast_to([P, REP, H2]))
                qf_t = wp.tile([P, JQ], F32, name=f"qf_{tt}", tag="qf", bufs=2)
                qf3 = qf_t.rearrange("p (h d) -> p h d", h=REP)
                nc.vector.tensor_mul(qf3, qn3,
                                     cos_t[:, None, :].broadcast_to([P, REP, HD]))
                qfr_t = wp.tile([P, JQ], F32R, name=f"qfr_{tt}", tag="qfr", bufs=3)
                nc.gpsimd.tensor_add(qfr_t, qf_t, qB_t)
                yield

                # k: rms norm + rope (rope on Pool)
                kn_t = wp.tile([P, HD], F32, name=f"kn_{tt}", tag="kn", bufs=2)
                nc.vector.tensor_scalar_mul(kn_t, kv_ps[:, 0:HD], rstdk)
                kB_t = wp.tile([P, HD], F32, name=f"kB_{tt}", tag="kB", bufs=2)
                nc.gpsimd.tensor_mul(kB_t[:, 0:H2], kn_t[:, H2:HD], sin_t[:, 0:H2])
                nc.gpsimd.tensor_mul(kB_t[:, H2:HD], kn_t[:, 0:H2], sin_t[:, H2:HD])
                kf_t = wp.tile([P, HD], F32, name=f"kf_{tt}", tag="kf", bufs=2)
                nc.gpsimd.tensor_mul(kf_t, kn_t, cos_t)
                kfr_t = wp.tile([P, HD], F32R, name=f"kfr_{tt}", tag="kfr", bufs=3)
                nc.gpsimd.tensor_add(kfr_t, kf_t, kB_t)
                # v: scale rows by token rstd
                with nc.allow_low_precision(reason="bf16 matmul operand"):
                    nc.vector.tensor_scalar_mul(vN[:, tt, :], kv_ps[:, HD:2 * HD],
                                                rstdx)
                yield

                prev_qk.append((qfr_t, kfr_t, tt))
                yield

            # ----------------- attention for one (chunk, head) --------------
            def attend(c, h, o_ps):
                nst = 4 * (c + 1)
                denf_a = wp.tile([P, TCW], F32R, name=f"dna_{c}_{h}",
                                 tag="dena", bufs=4)
                denf_b = None
                if c >= 1:
                    denf_b = wp.tile([P, TCW], F32R, name=f"dnb_{c}_{h}",
                                     tag="denb", bufs=4)
                for st in range(nst):
                    dv = st - 4 * c
                    off = dv * P if dv >= 0 else 0
                    w = TCW - off
                    sc = psp.tile([P, w], F32, name=f"sc_{c}_{h}_{st}",
                                  tag="mm", bufs=3)
                    nc.tensor.matmul(sc, kTt[:, st * P:(st + 1) * P],
                                     qT[h][:, c * TCW + off:(c + 1) * TCW],
                                     start=True, stop=True)
                    if dv >= 0:
                        nc.vector.tensor_add(sc[:, 0:P], sc[:, 0:P], mask_sb)
                    et = wp.tile([P, w], BF16, name=f"et_{c}_{h}_{st}",
                                 tag="et", bufs=8)
                    nc.scalar.activation(et, sc, AF.Exp, scale=SCALE)
                    if st == 0:
                        nc.vector.tensor_copy(denf_a, et)
                    elif c >= 1 and st == 1:
                        nc.vector.tensor_copy(denf_b, et)
                    elif c >= 1 and st % 2 == 1:
                        nc.gpsimd.tensor_add(denf_b[:, off:TCW],
                                             denf_b[:, off:TCW], et)
                    else:
                        nc.vector.tensor_add(denf_a[:, off:TCW],
                                             denf_a[:, off:TCW], et)
                    nc.tensor.matmul(o_ps[:, off:TCW], vN[:, st, :], et,
                                     start=(st == 0), stop=(st == nst - 1),
                                     skip_group_check=True)
                    yield
                # denominator: partition-sum both partials on the PE directly
                ds_ps = psp.tile([1, TCW], F32, name=f"ds_{c}_{h}",
                                 tag="mm", bufs=3)
                nc.tensor.matmul(ds_ps, ones_col, denf_a,
                                 start=True, stop=(c == 0))
                if c >= 1:
                    nc.tensor.matmul(ds_ps, ones_col, denf_b,
                                     start=False, stop=True)
                dsum = wp.tile([1, TCW], F32R, name=f"dsum_{c}_{h}",
                               tag="dsum", bufs=2)
                with nc.allow_low_precision(reason="fp32r matmul operand"):
                    nc.scalar.copy(dsum, ds_ps)
                rb_ps = psp.tile([P, TCW], F32, name=f"rb_{c}_{h}",
                                 tag="mm", bufs=3)
                nc.tensor.matmul(rb_ps, ones_row, dsum, start=True, stop=True)
                rb = wp.tile([P, TCW], F32, name=f"rbs_{c}_{h}", tag="rb", bufs=2)
                nc.vector.reciprocal_approx_fast(rb, rb_ps)
                aot = wp.tile([P, TCW], BF16, name=f"ao_{c}_{h}", tag="ao", bufs=8)
                with nc.allow_low_precision(reason="bf16 matmul operand"):
                    nc.vector.tensor_mul(aot, o_ps, rb)
                ao_tiles[(c, h)] = aot
                yield

            def chunk_gen(c):
                """Attention for chunk c: two head-pairs, interleaved. The
                last chunk runs all four heads concurrently, borrowing the
                q_ps PSUM banks that the finished qkv tile stream no longer
                needs."""
                if not wproj_loaded[0]:
                    load_wproj()
                if c == NTC - 1:
                    gens = []
                    for h in range(REP):
                        o = psp.tile([P, TCW], F32, name=f"ops_{c}_{h}",
                                     tag="acc" if h < 2 else "qps", bufs=2)
                        gens.append(attend(c, h, o))
                    alive = True
                    while alive:
                        alive = False
                        for g in gens:
                            try:
                                next(g)
                                alive = True
                            except StopIteration:
                                pass
                        yield
                    return
                for hp in (0, 2):
                    o0 = psp.tile([P, TCW], F32, name=f"ops_{c}_{hp}",
                                  tag="acc", bufs=2)
                    o1 = psp.tile([P, TCW], F32, name=f"ops_{c}_{hp + 1}",
                                  tag="acc", bufs=2)
                    g0 = attend(c, hp, o0)
                    g1 = attend(c, hp + 1, o1)
                    alive = True
                    while alive:
                        alive = False
                        for g in (g0, g1):
                            try:
                                next(g)
                                alive = True
                            except StopIteration:
                                pass
                        yield

            def proj_steps(c):
                """Projection for chunk c as small emit-steps (4 MMs each)."""
                for ttl in range(4):
                    yt = wp.tile([P, C], F32, name=f"y_{c}_{ttl}", tag="y", bufs=2)
                    for ncs in range(4):
                        y_ps = psp.tile([P, TCW], F32,
                                        name=f"yps_{c}_{ttl}_{ncs}",
                                        tag="mm", bufs=3)
                        for h in range(REP):
                            nc.tensor.matmul(
                                y_ps,
                                ao_tiles[(c, h)][:, ttl * P:(ttl + 1) * P],
                                wproj_sb[:, h, ncs * TCW:(ncs + 1) * TCW],
                                start=(h == 0), stop=(h == REP - 1))
                        # ACT helps only while qkv tiles still run; in the tail
                        # the scalar engine is exp-critical, so DVE evacuates
                        if c < 2 and ncs % 2 == 1:
                            nc.scalar.copy(yt[:, ncs * TCW:(ncs + 1) * TCW], y_ps)
                        else:
                            nc.vector.tensor_copy(yt[:, ncs * TCW:(ncs + 1) * TCW],
                                                  y_ps)
                        yield
                    row = (c * 4 + ttl) * P
                    nc.sync.dma_start(out=y_d.ap()[row:row + P, :], in_=yt)
                    yield

            # ----------------- master weave ---------------------------------
            def drain(g, n):
                for _ in range(n):
                    try:
                        next(g)
                    except StopIteration:
                        return False
                return True

            def tiles_stream():
                for tt in range(TT):
                    yield from tile_work(tt)
                while prev_qk:
                    args = prev_qk.pop(0)
                    emit_qk_transposes(*args)
                    tiles_emitted[0] = args[2] + 1
                    yield

            ts = tiles_stream()
            ts_alive = True
            attn_q = [0, 1, 2, 3]
            cur_attn = None
            cur_attn_c = None
            proj_q = []
            cur_proj = None
            while ts_alive or cur_attn or attn_q or cur_proj or proj_q:
                if ts_alive:
                    ts_alive = drain(ts, 2)
                if cur_attn is None and attn_q and \
                        tiles_emitted[0] >= 4 * (attn_q[0] + 1):
                    cur_attn_c = attn_q.pop(0)
                    cur_attn = chunk_gen(cur_attn_c)
                if cur_attn is not None:
                    if not drain(cur_attn, 1):
                        proj_q.append(cur_attn_c)
                        cur_attn = None
                if cur_proj is None and proj_q:
                    cur_proj = proj_steps(proj_q.pop(0))
                if cur_proj is not None:
                    if not drain(cur_proj, 1):
                        cur_proj = None

    return nc


_NC_CACHE = {}
LAST_RESULT = None


def _get_nc():
    if "v3" not in _NC_CACHE:
        nc = bacc.Bacc("TRN2", target_bir_lowering=False, debug=False)
        _emit(nc)
        nc.compile()
        _NC_CACHE["v3"] = nc
    return _NC_CACHE["v3"]


def _host_tables():
    inv_freq = 1.0 / (10000.0 ** (np.arange(0, HD, 2, dtype=np.float64) / HD))
    t = np.arange(T, dtype=np.float64)
    freqs = np.outer(t, inv_freq)                      # [T, 64]
    emb = np.concatenate([freqs, freqs], axis=-1)      # [T, 128]
    cos = np.cos(emb).astype(np.float32)
    sin = np.sin(emb).astype(np.float32)
    sin_signed = sin.copy()
    sin_signed[:, :HD // 2] *= -1.0                    # first half gets -sin
    # single diagonal-block causal mask, scoresT layout:
    # mask[s, t] = 0 if s <= t else -1e30 (within a 128x128 diagonal block)
    s = np.arange(P)[:, None]
    tcol = np.arange(P)[None, :]
    mask = np.where(s <= tcol, 0.0, MASKVAL).astype(np.float32)
    ident = np.eye(P, dtype=np.float32)
    return cos, sin_signed, mask, ident


def kernel(x, w_qkv, w_proj, q_gain):
    global LAST_RESULT
    x = np.asarray(x, dtype=np.float32)
    w_qkv = np.asarray(w_qkv, dtype=np.float32)
    w_proj = np.asarray(w_proj, dtype=np.float32)
    q_gain = np.asarray(q_gain, dtype=np.float32)

    cos, sin_signed, mask, ident = _host_tables()
    nc = _get_nc()

    x_bf = [np.ascontiguousarray(x[b].astype(ml_dtypes.bfloat16))
            for b in range(B)]
    xt_bf = [np.ascontiguousarray(x_bf[b].T) for b in range(B)]
    in_maps = []
    for r in range(8):
        b, g = r // 4, r % 4
        wq = w_qkv[:, g * JQ:(g + 1) * JQ]
        wk = w_qkv[:, C + g * HD:C + (g + 1) * HD]
        wv = w_qkv[:, C + KV_DIM + g * HD:C + KV_DIM + (g + 1) * HD]
        in_maps.append({
            "xb": x_bf[b],
            "xt": xt_bf[b],
            "wqkv": np.ascontiguousarray(
                np.concatenate([wq, wk, wv], axis=1)).astype(ml_dtypes.bfloat16),
            "wproj": np.ascontiguousarray(
                w_proj[g * JQ:(g + 1) * JQ, :]).astype(ml_dtypes.bfloat16),
            "gain": np.ascontiguousarray(q_gain[g * REP:(g + 1) * REP].reshape(1, REP)),
            "costab": cos,
            "sintab": sin_signed,
            "maskdiag": mask,
            "ident": ident,
        })

    trace = os.environ.get("KERNEL_TRACE") == "1"
    if trace:
        try:
            import antenv.axon_hooks  # noqa: F401
        except ImportError:
            trace = False
    res = run_bass_kernel_spmd(nc, in_maps, core_ids=list(range(8)), trace=trace)
    LAST_RESULT = res

    out = np.zeros((B, T, C), dtype=np.float32)
    for r in range(8):
        b = r // 4
        out[b] += res.results[r]["y"]
    return out
